# revision 33
# baseline (speedup 1.0000x reference)
"""Trainium2 Bass kernel for nn_BilinearPairedLayer.

out[b,i,j,o] = celu(zl[b,i] @ fc_l_W^T + fc_l_b) @ W[o] @ celu(zr[b,j] @ fc_r_W^T + fc_r_b) + bb[o]

with context-3 pairing:
  zl = [x_l, shift_fwd(x_l,1), shift_bwd(x_l,1)]   (192 features)
  zr = [x_l, shift_bwd(x_r,1), shift_fwd(x_r,1)]   (faithful torch-source bug: x_l first)

Shapes: B=2, N=512, n_in=64, H=128, n_out=8 -> out [2,512,512,8] f32.

Sharding: 8 cores = (b in {0,1}) x (j-chunk in {0..3} of 128 columns).
Each core computes out[b, :, j0:j0+128, :] (as bf16; host upcasts).

Per-core dataflow (contraction dims pre-transposed onto partitions host-side).

DMA strategy (empirically tuned against NTFF packet/semaphore data):
  - ALL inputs ride the sync HWDGE ring: one merged "hot" DMA first
    (D1w fc-weight overlay | D1x x-slices+biases | D1b xlhT) whose
    first-on-ring semaphore completes fast, then Wt.  (A 3-way split
    that lands the hr-critical block earlier A/B-measured neutral; the
    earlier compute start just runs more matmuls at the cold clock.)
  - The scalar ring carries only the ACT table load (pulled early by a
    dummy Exp).  Routing any output DMA over it regressed 3-4us.
  - Outputs are bf16 in FIVE DMAs on the sync ring (single chunk
    first/last, pairs between), each issued as soon as its eviction
    lands, so the 16 shared mover engines stream output packets while
    later main matmuls still run.  Each DMA writes its own DENSE DRAM
    tensor (slices of one big tensor write 2KB chunks at 8KB stride:
    ~185 GB/s vs ~310 dense).

Compute schedule:
  0. PE warm-up matmuls on memset tiles fill the ENTIRE input-DMA wait
     (~7.3us -> ~11.5us): the PE_HAM clock gate un-throttles (1.2 ->
     2.4 GHz) only after a ~3.4us fully-busy window of its free-running
     activity monitor, so the warm stream keeps the PE busy with no
     gaps until the input semaphore lands.  4 short (128-wide) warms
     start as early as possible off a tiny memset; a sacrificial
     256-wide transition matmul absorbs the ~220ns PE bubble that
     follows the stream's first shape change; 7 long (512-wide) warms
     carry to the DMA horizon.  4 keep-alive matmuls (lhsT = hrT pins
     them after v2) bridge the v2->main PE gap.
  1. fc biases are accumulated INTO PSUM by K=1 rank-1 matmuls
     (bias-row^T @ ones-row) that run FIRST in each accumulation group:
     celu = EXP (ACT, psum src) -> fused (-1,min 0) TS (DVE) -> max TT
     (DVE, psum operand), writing bf16.  hr first (feeds v2); hl in
     two 256-col groups whose celu pipelines overlap (separate psum +
     e_l tiles per group -- dep tracking is COARSE PER TILE, a shared
     tile would serialize group 1's EXP behind group 0's TS/TT).
  2. v2[h, j*8+o]: per o: WT_o.T @ hrT -> psum [h, o*128+j]; TWO
     strided casts (one per og group, on ACT after the hl EXPs) write
     the j-major/o-fast INTERLEAVED bf16 layout.  With this order,
     main-output partition p has o = p%8 for every chunk, so ONE
     shared [128,1] bias AP serves all evictions.
  3. TRANSPOSED main: psum[jo-block, i] = v2_c.T @ hlT, 8 matmuls N=512
     into the PSUM map documented at the tile declarations: every main
     chunk's tile retires its pre-main tenant (warms/hr/v2-og/hl-G)
     before the chunk lands, and no eviction shares a tile with a
     later writer, so no coarse-tile dep ever stalls the pipeline.
  4. Evictions are fused copy+bias+bf16-cast ops sized to their DMA
     ([512] singles, [1024] pairs), alternating ACT (activation-with-
     bias) and DVE (tensor_scalar), chasing the main matmul
     completions; each DMA is issued immediately after its eviction.

walrus's per-instruction HW structs carry at most ONE sync wait; a post-pass
splits multi-wait instructions into single-wait EventSemaphore predecessors.
"""

import numpy as np

import concourse.bass as bass
import concourse.mybir as mybir
import concourse.tile as tile
from concourse.bass_utils import run_bass_kernel_spmd

F32 = mybir.dt.float32
BF16 = mybir.dt.bfloat16

B = 2
N = 512
NIN = 64
H = 128
O = 8
JC = 128  # j-chunk per core
N_CORES = 8

# D1x packed-column offsets (bf16 elements)
_XLJ = 0              # xljT  [128]   (rows 0:64)
_XRH = 128            # xrhT  [130]   (rows 0:64)
_BRR = 258            # fc_r_b as a row on partition 0  [128]
_BLR = 386            # fc_l_b as a row on partition 0  [128]
_OBI = 514            # out-bias bb[p%8] per partition (f32 bitcast, 2 cols)
_D1XW = 516
_D1BW = 514           # xlhT (rows 64:128)

N_WARM_SHORT = 4      # 128-wide, start off the tiny wsA memset
N_WARM_LONG = 7       # 512-wide, carry PE activity to the DMA horizon


def build_nc():
    nc = bass.Bass("TRN2")

    Dh = nc.dram_tensor("Dh", [128, 3 * H + _D1XW + _D1BW], BF16,
                        kind="ExternalInput")
    Wt = nc.dram_tensor("Wt", [128, O * H], BF16, kind="ExternalInput")
    # separate dense DRAM tensors per output DMA (see docstring);
    # first and last are single chunks so the packet stream starts as
    # early as possible and the tail DMA is short.
    _OW = [512, 1024, 1024, 1024, 512]
    outs = [nc.dram_tensor(f"out{i}", [128, w], BF16,
                           kind="ExternalOutput") for i, w in enumerate(_OW)]

    with tile.TileContext(nc) as tc:
        with (
            tc.tile_pool(name="persist", bufs=1) as pp,
            tc.tile_pool(name="psum", bufs=1, space="PSUM") as psp,
        ):
            Dh_sb = pp.tile([128, 3 * H + _D1XW + _D1BW], BF16, name="Dh_sb")
            W_sb = pp.tile([128, O * H], BF16, name="W_sb")
            warm_sb = pp.tile([128, 640], BF16, name="warm_sb")
            ones_sb = pp.tile([1, N], BF16, name="ones_sb")
            hrT = pp.tile([128, JC], BF16, name="hrT")
            wsA = warm_sb[:, 0:128]
            wsB = warm_sb[:, 128:640]

            # ---- ALL inputs on the sync ring: the hot layer-1 block
            # (incl. D1b) first so it drains at full rate with a fast
            # first-on-ring sem; Wt behind it.
            nc.sync.dma_start(Dh_sb[:], Dh[:])
            nc.sync.dma_start(W_sb[:], Wt[:])

            # ---- warm-tile memsets (short lhs first so warms start
            # ASAP) + early ACT table load via a dummy Exp (dst = hrT
            # cell: only a WAW dep against the much-later celu TT).
            nc.vector.memset(wsA, 0.0)
            nc.vector.memset(wsB, 0.0)
            nc.vector.memset(ones_sb[:], 1.0)
            nc.scalar.activation(hrT[0:1, 0:1], warm_sb[0:1, 0:1],
                                 mybir.ActivationFunctionType.Exp)

            # PSUM map (4096 f32 cols, exactly full).  Dep tracking is
            # COARSE PER TILE: a reader waits the tile's LAST
            # earlier-emitted write, a writer waits ALL earlier reads
            # of the tile.  The layout below is chosen so every
            # implicit tile-level dep is one the schedule satisfies:
            #   S0  [512]: warm-ups | hr layer-1 [0:128] | keep-alives
            #              | main c0.  (hr-celu reads finish long
            #              before c0; e0 evicts after c0.)
            #   P12 [1024]: v2 og0 [0:512] | mains c1, c2.  (cast0
            #              reads og0; c1/c2 overwrite after the cast.)
            #   P34 [1024]: v2 og1 [0:512] | mains c3, c4.
            #   P56 [1024]: hl psum [0:512] | mains c5, c6.
            #   S7  [512]: main c7 only (pristine: no eviction or
            #              pre-main entanglement).
            S0 = psp.tile([128, 512], F32, name="S0")
            P12 = psp.tile([128, 1024], F32, name="P12")
            P34 = psp.tile([128, 1024], F32, name="P34")
            P56 = psp.tile([128, 1024], F32, name="P56")
            S7 = psp.tile([128, 512], F32, name="S7")
            ps_hr = S0[:, 0:128]
            ps_v0 = P12[:, 0:512]
            ps_v1 = P34[:, 0:512]
            ps_hl = P56[:, 0:512]

            # ---- PE warm-up stream: no gaps until the input DMA lands.
            # The first shape-change in the stream costs a ~220ns PE
            # bubble (observed after the first wide matmul on every
            # run); a sacrificial 256-wide transition matmul moves that
            # bubble to the start of the stream where it breaks fewer
            # HAM activity windows.
            for _ in range(N_WARM_SHORT):
                nc.tensor.matmul(S0[:, 0:128], wsA, wsA,
                                 start=True, stop=True)
            nc.tensor.matmul(S0[:, 0:256], wsA, wsB[:, 0:256],
                             start=True, stop=True)
            for _ in range(N_WARM_LONG):
                nc.tensor.matmul(S0[:], wsA, wsB,
                                 start=True, stop=True)

            # ---- layer 1 matmuls; K=1 bias matmul runs FIRST ----
            nc.tensor.matmul(ps_hr, Dh_sb[0:1, 3 * H + _BRR:3 * H + _BRR + H],
                             ones_sb[0:1, 0:JC], start=True, stop=False)
            xo = 3 * H
            rhs_r = [
                Dh_sb[:, xo + _XLJ:xo + _XLJ + JC],          # x_l[j]
                Dh_sb[:, xo + _XRH + 2:xo + _XRH + 2 + JC],  # x_r[j+1]
                Dh_sb[:, xo + _XRH:xo + _XRH + JC],          # x_r[j-1]
            ]
            for c in range(3):
                nc.tensor.matmul(
                    ps_hr, Dh_sb[:, c * H:(c + 1) * H],
                    rhs_r[c], start=False, stop=(c == 2),
                )

            # hl layer-1 as ONE 512-wide group: fewer matmuls (less
            # fixed overhead, matters at the cold clock) and one EXP
            # instead of two 256-wide ones (-262ns of ACT-queue time,
            # which gates main-start via the casts).
            xb = 3 * H + _D1XW
            nc.tensor.matmul(ps_hl,
                             Dh_sb[0:1, 3 * H + _BLR:3 * H + _BLR + H],
                             ones_sb[0:1, 0:N], start=True, stop=False)
            rhs_l = [
                Dh_sb[:, xb + 1:xb + 1 + N],    # x_l[i]
                Dh_sb[:, xb + 0:xb + N],        # x_l[i-1]
                Dh_sb[:, xb + 2:xb + 2 + N],    # x_l[i+1]
            ]
            for c in range(3):
                nc.tensor.matmul(
                    ps_hl, Dh_sb[:, c * H:(c + 1) * H],
                    rhs_l[c], start=False, stop=(c == 2),
                )

            # ---- hr celu: e (ACT) -> TS min (DVE) -> TT max (DVE) ----
            e_r = pp.tile([128, JC], F32, name="e_r")
            nc.scalar.activation(e_r[:], ps_hr,
                                 mybir.ActivationFunctionType.Exp)
            nc.vector.tensor_scalar(e_r[:], e_r[:], -1.0, 0.0,
                                    mybir.AluOpType.add,
                                    mybir.AluOpType.min)
            nc.vector.tensor_tensor(hrT[:], ps_hr, e_r[:],
                                    mybir.AluOpType.max)

            # ---- hl celu (full width: one EXP/TS/TT) ----
            hlT = pp.tile([128, N], BF16, name="hlT")
            e_l = pp.tile([128, N], F32, name="e_l")
            nc.scalar.activation(e_l[:], ps_hl,
                                 mybir.ActivationFunctionType.Exp)
            nc.vector.tensor_scalar(e_l[:], e_l[:], -1.0, 0.0,
                                    mybir.AluOpType.add,
                                    mybir.AluOpType.min)
            nc.vector.tensor_tensor(hlT[:], ps_hl,
                                    e_l[:], mybir.AluOpType.max)

            # ---- v2 matmuls: psum [h, (o,j)] per og group ----
            for ps_vo, o0 in ((ps_v0, 0), (ps_v1, 4)):
                for ol in range(4):
                    o = o0 + ol
                    nc.tensor.matmul(
                        ps_vo[:, ol * JC:(ol + 1) * JC],
                        W_sb[:, o * H:(o + 1) * H], hrT[:],
                        start=True, stop=True,
                    )

            # HAM keep-alive: bridge the PE idle window between v2 and
            # main.  lhsT = hrT pins the dependency so the scheduler
            # cannot hoist these before the layer-1/v2 matmuls.
            for _ in range(4):
                nc.tensor.matmul(
                    S0[:, 0:256], hrT, warm_sb[:, 128:384],
                    start=True, stop=True,
                )

            # ---- v2 casts to interleaved bf16 layout (col = j*8+o),
            # one per og group: og0/og1 live in different psum tiles,
            # so cast0 starts as soon as og0's 4 matmuls land.
            v2sb = pp.tile([128, O * H], BF16, name="v2sb")
            v2v = v2sb[:].rearrange("p (j g o) -> p j g o", g=2, o=4)
            nc.scalar.copy(v2v[:, :, 0, :],
                           ps_v0.rearrange("p (o j) -> p j o", o=4))
            nc.scalar.copy(v2v[:, :, 1, :],
                           ps_v1.rearrange("p (o j) -> p j o", o=4))

            # ---- main (transposed): psum[jo-block, i] = v2_c.T @ hlT ----
            # chunk c partition p -> j = 16c + p//8, o = p%8
            main_dst = [
                S0[:], P12[:, 0:512], P12[:, 512:1024],
                P34[:, 0:512], P34[:, 512:1024],
                P56[:, 0:512], P56[:, 512:1024], S7[:],
            ]
            for c in range(8):
                nc.tensor.matmul(
                    main_dst[c], v2sb[:, c * JC:(c + 1) * JC], hlT[:],
                    start=True, stop=True,
                )

            # separate staging tiles per output DMA: out_sb as ONE tile
            # would serialize the evictions via coarse WAW tracking.
            obs = [pp.tile([128, w], BF16, name=f"ob{i}")
                   for i, w in enumerate(_OW)]
            ob_ap = Dh_sb[:, 3 * H + _OBI:3 * H + _OBI + 2].bitcast(F32)

            def evict(eng, src, dst):
                if eng is nc.scalar:
                    nc.scalar.activation(dst, src,
                                         mybir.ActivationFunctionType.Identity,
                                         bias=ob_ap, scale=1.0)
                else:
                    nc.vector.tensor_scalar_add(dst, src, ob_ap)

            # pair evictions chase the main matmuls; each DMA issues
            # right after its eviction.  ACT: e0, e34, e7; DVE: e12, e56.
            evict(nc.scalar, S0[:], obs[0][:])            # e0  (c0)
            nc.sync.dma_start(outs[0][:], obs[0][:])
            evict(nc.vector, P12[:], obs[1][:])           # e12 (c1,c2)
            nc.sync.dma_start(outs[1][:], obs[1][:])
            evict(nc.scalar, P34[:], obs[2][:])           # e34 (c3,c4)
            nc.sync.dma_start(outs[2][:], obs[2][:])
            evict(nc.vector, P56[:], obs[3][:])           # e56 (c5,c6)
            nc.sync.dma_start(outs[3][:], obs[3][:])
            evict(nc.scalar, S7[:], obs[4][:])            # e7  (c7)
            nc.sync.dma_start(outs[4][:], obs[4][:])

    _legalize_waits(nc)
    return nc


def _legalize_waits(nc):
    """walrus's per-instruction HW structs carry at most ONE sync wait.
    Split any instruction with >1 on_wait into same-engine single-wait
    EventSemaphore predecessors (engine executes them in program order)."""
    n = 0
    for bb in nc.main_func.blocks:
        insts = list(bb.instructions)
        out = []
        for ins in insts:
            si = ins.sync_info
            waits = list(si.on_wait) if si and si.on_wait else []
            if len(waits) > 1:
                for w in waits[:-1]:
                    n += 1
                    out.append(mybir.InstEventSemaphore(
                        name=f"wait-split-{n}",
                        opcode="EventSemaphore",
                        engine=ins.engine,
                        ins=[], outs=[],
                        sync_info=mybir.SyncInfo(on_wait=[w], on_update=[]),
                    ))
                si.on_wait = [waits[-1]]
            out.append(ins)
        if n:
            bb.instructions = out
    return nc


_NC_CACHE = None


def _get_nc():
    global _NC_CACHE
    if _NC_CACHE is None:
        _NC_CACHE = build_nc()
    return _NC_CACHE


def _prep_core_inputs(x_l, x_r, fc_l_W, fc_l_b, fc_r_W, fc_r_b, bilinear_W, bilinear_b):
    """Host-side sharding: build the 8 per-core input dicts."""
    import ml_dtypes

    f32 = np.float32
    bf16 = ml_dtypes.bfloat16
    x_l = np.ascontiguousarray(x_l, f32)
    x_r = np.ascontiguousarray(x_r, f32)

    # WT[g, o*H + h] = W[o, h, g]
    WT = np.ascontiguousarray(
        np.asarray(bilinear_W, f32).transpose(2, 0, 1).reshape(128, O * H)
    ).astype(bf16)

    D1w = np.zeros((128, 3 * H), bf16)
    frW = np.asarray(fc_r_W, f32)
    flW = np.asarray(fc_l_W, f32)
    for c in range(3):
        D1w[:NIN, c * H:(c + 1) * H] = frW[:, c * NIN:(c + 1) * NIN].T.astype(bf16)
        D1w[NIN:, c * H:(c + 1) * H] = flW[:, c * NIN:(c + 1) * NIN].T.astype(bf16)

    D1x_c = np.zeros((128, _D1XW), bf16)
    D1x_c[0, _BRR:_BRR + H] = np.asarray(fc_r_b, f32).astype(bf16)
    D1x_c[0, _BLR:_BLR + H] = np.asarray(fc_l_b, f32).astype(bf16)
    obi = np.asarray(bilinear_b, f32)[np.arange(128) % O]  # bb[p%8]
    D1x_c.view(np.uint16)[:, _OBI:_OBI + 2] = obi.reshape(-1, 1).view('<u2')

    # D1b per batch: xlhT rows 64:128, col t = x_l[b, t-1]
    D1bs = []
    for b in range(B):
        D1b = np.zeros((128, _D1BW), bf16)
        D1b[NIN:, 1:1 + N] = x_l[b].T.astype(bf16)
        D1bs.append(D1b)

    in_maps = []
    for core in range(N_CORES):
        b, jg = core // 4, core % 4
        j0 = jg * JC
        D1x = D1x_c.copy()
        D1x[:NIN, _XLJ:_XLJ + JC] = x_l[b, j0:j0 + JC].T.astype(bf16)
        # xrhT: col t = x_r[b, j0-1+t], zero-padded at global edges
        lo = max(j0 - 1, 0)
        hi = min(j0 + JC + 1, N)
        D1x[:NIN, _XRH + lo - (j0 - 1):_XRH + hi - (j0 - 1)] = \
            x_r[b, lo:hi].T.astype(bf16)
        in_maps.append({
            "Dh": np.concatenate([D1w, D1x, D1bs[b]], axis=1),
            "Wt": WT,
        })
    return in_maps


def _run(inputs, trace=False, **kw):
    nc = _get_nc()
    in_maps = _prep_core_inputs(**inputs)
    res = run_bass_kernel_spmd(
        nc, in_maps, core_ids=list(range(N_CORES)), trace=trace, **kw)
    out = np.empty((B, N, N, O), np.float32)
    for core in range(N_CORES):
        b, jg = core // 4, core % 4
        j0 = jg * JC
        # device out: [p = jr*8+o, c*512 + i] -> out[i, 16c+jr, o]
        r = res.results[core]
        arr = np.concatenate(
            [np.asarray(r[f"out{i}"]) for i in range(5)],
            axis=1).astype(np.float32)
        arr = arr.reshape(16, 8, 8, N)          # [jr, o, c, i]
        out[b, :, j0:j0 + JC, :] = \
            arr.transpose(3, 2, 0, 1).reshape(N, JC, O)
    return out, res


def kernel(**inputs):
    out, _ = _run(inputs, trace=False)
    return out


# revision 34
# speedup vs baseline: 1.0523x; 1.0523x over previous
"""Trainium2 Bass kernel for nn_BilinearPairedLayer.

out[b,i,j,o] = celu(zl[b,i] @ fc_l_W^T + fc_l_b) @ W[o] @ celu(zr[b,j] @ fc_r_W^T + fc_r_b) + bb[o]

with context-3 pairing:
  zl = [x_l, shift_fwd(x_l,1), shift_bwd(x_l,1)]   (192 features)
  zr = [x_l, shift_bwd(x_r,1), shift_fwd(x_r,1)]   (faithful torch-source bug: x_l first)

Shapes: B=2, N=512, n_in=64, H=128, n_out=8 -> out [2,512,512,8] f32.

Sharding: 8 cores = (b in {0,1}) x (j-chunk in {0..3} of 128 columns).
Each core computes out[b, :, j0:j0+128, :] (as bf16; host upcasts).

Per-core dataflow (contraction dims pre-transposed onto partitions host-side).

DMA strategy (empirically tuned against NTFF packet/semaphore data):
  - ALL inputs ride the sync HWDGE ring: one merged "hot" DMA first
    (D1w fc-weight overlay | D1x x-slices+biases | D1b xlhT) whose
    first-on-ring semaphore completes fast, then Wt.  (A 3-way split
    that lands the hr-critical block earlier A/B-measured neutral; the
    earlier compute start just runs more matmuls at the cold clock.)
  - The scalar ring carries only the ACT table load (pulled early by a
    dummy Exp).  Routing any output DMA over it regressed 3-4us.
  - Outputs are bf16 in FIVE DMAs on the sync ring (single chunk
    first/last, pairs between), each issued as soon as its eviction
    lands, so the 16 shared mover engines stream output packets while
    later main matmuls still run.  Each DMA writes its own DENSE DRAM
    tensor (slices of one big tensor write 2KB chunks at 8KB stride:
    ~185 GB/s vs ~310 dense).

Compute schedule:
  0. PE warm-up matmuls on memset tiles fill the ENTIRE input-DMA wait
     (~7.3us -> ~11.5us): the PE_HAM clock gate un-throttles (1.2 ->
     2.4 GHz) only after a ~3.4us fully-busy window of its free-running
     activity monitor, so the warm stream keeps the PE busy with no
     gaps until the input semaphore lands.  4 short (128-wide) warms
     start as early as possible off a tiny memset; a sacrificial
     256-wide transition matmul absorbs the ~220ns PE bubble that
     follows the stream's first shape change; 7 long (512-wide) warms
     carry to the DMA horizon.  4 keep-alive matmuls (lhsT = hrT pins
     them after v2) bridge the v2->main PE gap.
  1. fc biases are accumulated INTO PSUM by K=1 rank-1 matmuls
     (bias-row^T @ ones-row) that run FIRST in each accumulation group:
     celu = EXP (ACT, psum src) -> fused (-1,min 0) TS (DVE) -> max TT
     (DVE, psum operand), writing bf16.  hr first (feeds v2); hl in
     two 256-col groups whose celu pipelines overlap (separate psum +
     e_l tiles per group -- dep tracking is COARSE PER TILE, a shared
     tile would serialize group 1's EXP behind group 0's TS/TT).
  2. v2[h, j*8+o]: per o: WT_o.T @ hrT -> psum [h, o*128+j]; TWO
     strided casts (one per og group, on ACT after the hl EXPs) write
     the j-major/o-fast INTERLEAVED bf16 layout.  With this order,
     main-output partition p has o = p%8 for every chunk, so ONE
     shared [128,1] bias AP serves all evictions.
  3. TRANSPOSED main: psum[jo-block, i] = v2_c.T @ hlT, 8 matmuls N=512
     into the PSUM map documented at the tile declarations: every main
     chunk's tile retires its pre-main tenant (warms/hr/v2-og/hl-G)
     before the chunk lands, and no eviction shares a tile with a
     later writer, so no coarse-tile dep ever stalls the pipeline.
  4. Evictions are fused copy+bias+bf16-cast ops sized to their DMA
     ([512] singles, [1024] pairs), alternating ACT (activation-with-
     bias) and DVE (tensor_scalar), chasing the main matmul
     completions; each DMA is issued immediately after its eviction.

walrus's per-instruction HW structs carry at most ONE sync wait; a post-pass
splits multi-wait instructions into single-wait EventSemaphore predecessors.
"""

import numpy as np

import concourse.bass as bass
import concourse.mybir as mybir
import concourse.tile as tile
from concourse.bass_utils import run_bass_kernel_spmd

F32 = mybir.dt.float32
BF16 = mybir.dt.bfloat16

B = 2
N = 512
NIN = 64
H = 128
O = 8
JC = 128  # j-chunk per core
N_CORES = 8

# D1x packed-column offsets (bf16 elements)
_XLJ = 0              # xljT  [128]   (rows 0:64)
_XRH = 128            # xrhT  [130]   (rows 0:64)
_BRR = 258            # fc_r_b as a row on partition 0  [128]
_BLR = 386            # fc_l_b as a row on partition 0  [128]
_OBI = 514            # out-bias bb[p%8] per partition (f32 bitcast, 2 cols)
_D1XW = 516
_D1BW = 514           # xlhT (rows 64:128)

N_WARM_SHORT = 4      # 128-wide, start off the tiny wsA memset
N_WARM_LONG = 7       # 512-wide, carry PE activity to the DMA horizon


def build_nc():
    nc = bass.Bass("TRN2")

    Dh = nc.dram_tensor("Dh", [128, 3 * H + _D1XW + _D1BW], BF16,
                        kind="ExternalInput")
    Wt = nc.dram_tensor("Wt", [128, O * H], BF16, kind="ExternalInput")
    # separate dense DRAM tensors per output DMA (see docstring);
    # first and last are single chunks so the packet stream starts as
    # early as possible and the tail DMA is short.
    _OW = [512, 1024, 1024, 1024, 512]
    outs = [nc.dram_tensor(f"out{i}", [128, w], BF16,
                           kind="ExternalOutput") for i, w in enumerate(_OW)]

    with tile.TileContext(nc) as tc:
        with (
            tc.tile_pool(name="persist", bufs=1) as pp,
            tc.tile_pool(name="psum", bufs=1, space="PSUM") as psp,
        ):
            Dh_sb = pp.tile([128, 3 * H + _D1XW + _D1BW], BF16, name="Dh_sb")
            W_sb = pp.tile([128, O * H], BF16, name="W_sb")
            warm_sb = pp.tile([128, 640], BF16, name="warm_sb")
            ones_sb = pp.tile([1, N], BF16, name="ones_sb")
            hrT = pp.tile([128, JC], BF16, name="hrT")
            wsA = warm_sb[:, 0:128]
            wsB = warm_sb[:, 128:640]

            # ---- ALL inputs on the sync ring: the hot layer-1 block
            # (incl. D1b) first so it drains at full rate with a fast
            # first-on-ring sem; Wt behind it.
            nc.sync.dma_start(Dh_sb[:], Dh[:])
            nc.sync.dma_start(W_sb[:], Wt[:])

            # ---- warm-tile memsets (short lhs first so warms start
            # ASAP) + early ACT table load via a dummy Exp (dst = hrT
            # cell: only a WAW dep against the much-later celu TT).
            nc.vector.memset(wsA, 0.0)
            nc.vector.memset(wsB, 0.0)
            nc.vector.memset(ones_sb[:], 1.0)
            nc.scalar.activation(hrT[0:1, 0:1], warm_sb[0:1, 0:1],
                                 mybir.ActivationFunctionType.Exp)

            # PSUM map (4096 f32 cols, exactly full).  Dep tracking is
            # COARSE PER TILE: a reader waits the tile's LAST
            # earlier-emitted write, a writer waits ALL earlier reads
            # of the tile.  The layout below is chosen so every
            # implicit tile-level dep is one the schedule satisfies:
            #   S0  [512]: warm-ups | hr layer-1 [0:128] | keep-alives
            #              | main c0.  (hr-celu reads finish long
            #              before c0; e0 evicts after c0.)
            #   P12 [1024]: v2 og0 [0:512] | mains c1, c2.  (cast0
            #              reads og0; c1/c2 overwrite after the cast.)
            #   P34 [1024]: v2 og1 [0:512] | mains c3, c4.
            #   P56 [1024]: hl psum [0:512] | mains c5, c6.
            #   S7  [512]: main c7 only (pristine: no eviction or
            #              pre-main entanglement).
            S0 = psp.tile([128, 512], F32, name="S0")
            P12 = psp.tile([128, 1024], F32, name="P12")
            P34 = psp.tile([128, 1024], F32, name="P34")
            P56 = psp.tile([128, 1024], F32, name="P56")
            S7 = psp.tile([128, 512], F32, name="S7")
            ps_hr = S0[:, 0:128]
            ps_v0 = P12[:, 0:512]
            ps_v1 = P34[:, 0:512]
            ps_hl = P56[:, 0:512]

            # ---- PE warm-up stream: no gaps until the input DMA lands.
            # The first shape-change in the stream costs a ~220ns PE
            # bubble (observed after the first wide matmul on every
            # run); a sacrificial 256-wide transition matmul moves that
            # bubble to the start of the stream where it breaks fewer
            # HAM activity windows.
            for _ in range(N_WARM_SHORT):
                nc.tensor.matmul(S0[:, 0:128], wsA, wsA,
                                 start=True, stop=True)
            nc.tensor.matmul(S0[:, 0:256], wsA, wsB[:, 0:256],
                             start=True, stop=True)
            for _ in range(N_WARM_LONG):
                nc.tensor.matmul(S0[:], wsA, wsB,
                                 start=True, stop=True)

            # ---- layer 1 matmuls; K=1 bias matmul runs FIRST ----
            nc.tensor.matmul(ps_hr, Dh_sb[0:1, 3 * H + _BRR:3 * H + _BRR + H],
                             ones_sb[0:1, 0:JC], start=True, stop=False)
            xo = 3 * H
            rhs_r = [
                Dh_sb[:, xo + _XLJ:xo + _XLJ + JC],          # x_l[j]
                Dh_sb[:, xo + _XRH + 2:xo + _XRH + 2 + JC],  # x_r[j+1]
                Dh_sb[:, xo + _XRH:xo + _XRH + JC],          # x_r[j-1]
            ]
            for c in range(3):
                nc.tensor.matmul(
                    ps_hr, Dh_sb[:, c * H:(c + 1) * H],
                    rhs_r[c], start=False, stop=(c == 2),
                )

            # hl layer-1 as ONE 512-wide group: fewer matmuls (less
            # fixed overhead, matters at the cold clock) and one EXP
            # instead of two 256-wide ones (-262ns of ACT-queue time,
            # which gates main-start via the casts).
            xb = 3 * H + _D1XW
            nc.tensor.matmul(ps_hl,
                             Dh_sb[0:1, 3 * H + _BLR:3 * H + _BLR + H],
                             ones_sb[0:1, 0:N], start=True, stop=False)
            rhs_l = [
                Dh_sb[:, xb + 1:xb + 1 + N],    # x_l[i]
                Dh_sb[:, xb + 0:xb + N],        # x_l[i-1]
                Dh_sb[:, xb + 2:xb + 2 + N],    # x_l[i+1]
            ]
            for c in range(3):
                nc.tensor.matmul(
                    ps_hl, Dh_sb[:, c * H:(c + 1) * H],
                    rhs_l[c], start=False, stop=(c == 2),
                )

            # ---- hr celu: e (ACT) -> TS min (DVE) -> TT max (DVE) ----
            e_r = pp.tile([128, JC], F32, name="e_r")
            nc.scalar.activation(e_r[:], ps_hr,
                                 mybir.ActivationFunctionType.Exp)
            nc.vector.tensor_scalar(e_r[:], e_r[:], -1.0, 0.0,
                                    mybir.AluOpType.add,
                                    mybir.AluOpType.min)
            nc.vector.tensor_tensor(hrT[:], ps_hr, e_r[:],
                                    mybir.AluOpType.max)

            # ---- hl celu (full width: one EXP/TS/TT) ----
            hlT = pp.tile([128, N], BF16, name="hlT")
            e_l = pp.tile([128, N], F32, name="e_l")
            nc.scalar.activation(e_l[:], ps_hl,
                                 mybir.ActivationFunctionType.Exp)
            nc.vector.tensor_scalar(e_l[:], e_l[:], -1.0, 0.0,
                                    mybir.AluOpType.add,
                                    mybir.AluOpType.min)
            nc.vector.tensor_tensor(hlT[:], ps_hl,
                                    e_l[:], mybir.AluOpType.max)

            # ---- v2 matmuls: psum [h, (o,j)] per og group ----
            for ps_vo, o0 in ((ps_v0, 0), (ps_v1, 4)):
                for ol in range(4):
                    o = o0 + ol
                    nc.tensor.matmul(
                        ps_vo[:, ol * JC:(ol + 1) * JC],
                        W_sb[:, o * H:(o + 1) * H], hrT[:],
                        start=True, stop=True,
                    )

            # HAM keep-alive: bridge the PE idle window between v2 and
            # main.  lhsT = hrT pins the dependency so the scheduler
            # cannot hoist these before the layer-1/v2 matmuls.
            for _ in range(4):
                nc.tensor.matmul(
                    S0[:, 0:256], hrT, warm_sb[:, 128:384],
                    start=True, stop=True,
                )

            # ---- v2 casts to interleaved bf16 layout (col = j*8+o),
            # one per og group: og0/og1 live in different psum tiles,
            # so cast0 starts as soon as og0's 4 matmuls land.
            v2sb = pp.tile([128, O * H], BF16, name="v2sb")
            v2v = v2sb[:].rearrange("p (j g o) -> p j g o", g=2, o=4)
            nc.scalar.copy(v2v[:, :, 0, :],
                           ps_v0.rearrange("p (o j) -> p j o", o=4))
            nc.scalar.copy(v2v[:, :, 1, :],
                           ps_v1.rearrange("p (o j) -> p j o", o=4))

            # ---- main (transposed): psum[jo-block, i] = v2_c.T @ hlT ----
            # chunk c partition p -> j = 16c + p//8, o = p%8
            main_dst = [
                S0[:], P12[:, 0:512], P12[:, 512:1024],
                P34[:, 0:512], P34[:, 512:1024],
                P56[:, 0:512], P56[:, 512:1024], S7[:],
            ]
            for c in range(8):
                nc.tensor.matmul(
                    main_dst[c], v2sb[:, c * JC:(c + 1) * JC], hlT[:],
                    start=True, stop=True,
                )

            # separate staging tiles per output DMA: out_sb as ONE tile
            # would serialize the evictions via coarse WAW tracking.
            obs = [pp.tile([128, w], BF16, name=f"ob{i}")
                   for i, w in enumerate(_OW)]
            ob_ap = Dh_sb[:, 3 * H + _OBI:3 * H + _OBI + 2].bitcast(F32)

            def evict(eng, src, dst):
                if eng is nc.scalar:
                    nc.scalar.activation(dst, src,
                                         mybir.ActivationFunctionType.Identity,
                                         bias=ob_ap, scale=1.0)
                else:
                    nc.vector.tensor_scalar_add(dst, src, ob_ap)

            # pair evictions chase the main matmuls; each DMA issues
            # right after its eviction.  ACT takes the pair evicts
            # (1114ns vs DVE's 1283 and free right after the casts);
            # DVE takes the cheap singles + e34: ACT e12, e56;
            # DVE e0, e34, e7.
            evict(nc.vector, S0[:], obs[0][:])            # e0  (c0)
            nc.sync.dma_start(outs[0][:], obs[0][:])
            evict(nc.scalar, P12[:], obs[1][:])           # e12 (c1,c2)
            nc.sync.dma_start(outs[1][:], obs[1][:])
            evict(nc.vector, P34[:], obs[2][:])           # e34 (c3,c4)
            nc.sync.dma_start(outs[2][:], obs[2][:])
            evict(nc.scalar, P56[:], obs[3][:])           # e56 (c5,c6)
            nc.sync.dma_start(outs[3][:], obs[3][:])
            evict(nc.vector, S7[:], obs[4][:])            # e7  (c7)
            nc.sync.dma_start(outs[4][:], obs[4][:])

    _legalize_waits(nc)
    return nc


def _legalize_waits(nc):
    """walrus's per-instruction HW structs carry at most ONE sync wait.
    Split any instruction with >1 on_wait into same-engine single-wait
    EventSemaphore predecessors (engine executes them in program order)."""
    n = 0
    for bb in nc.main_func.blocks:
        insts = list(bb.instructions)
        out = []
        for ins in insts:
            si = ins.sync_info
            waits = list(si.on_wait) if si and si.on_wait else []
            if len(waits) > 1:
                for w in waits[:-1]:
                    n += 1
                    out.append(mybir.InstEventSemaphore(
                        name=f"wait-split-{n}",
                        opcode="EventSemaphore",
                        engine=ins.engine,
                        ins=[], outs=[],
                        sync_info=mybir.SyncInfo(on_wait=[w], on_update=[]),
                    ))
                si.on_wait = [waits[-1]]
            out.append(ins)
        if n:
            bb.instructions = out
    return nc


_NC_CACHE = None


def _get_nc():
    global _NC_CACHE
    if _NC_CACHE is None:
        _NC_CACHE = build_nc()
    return _NC_CACHE


def _prep_core_inputs(x_l, x_r, fc_l_W, fc_l_b, fc_r_W, fc_r_b, bilinear_W, bilinear_b):
    """Host-side sharding: build the 8 per-core input dicts."""
    import ml_dtypes

    f32 = np.float32
    bf16 = ml_dtypes.bfloat16
    x_l = np.ascontiguousarray(x_l, f32)
    x_r = np.ascontiguousarray(x_r, f32)

    # WT[g, o*H + h] = W[o, h, g]
    WT = np.ascontiguousarray(
        np.asarray(bilinear_W, f32).transpose(2, 0, 1).reshape(128, O * H)
    ).astype(bf16)

    D1w = np.zeros((128, 3 * H), bf16)
    frW = np.asarray(fc_r_W, f32)
    flW = np.asarray(fc_l_W, f32)
    for c in range(3):
        D1w[:NIN, c * H:(c + 1) * H] = frW[:, c * NIN:(c + 1) * NIN].T.astype(bf16)
        D1w[NIN:, c * H:(c + 1) * H] = flW[:, c * NIN:(c + 1) * NIN].T.astype(bf16)

    D1x_c = np.zeros((128, _D1XW), bf16)
    D1x_c[0, _BRR:_BRR + H] = np.asarray(fc_r_b, f32).astype(bf16)
    D1x_c[0, _BLR:_BLR + H] = np.asarray(fc_l_b, f32).astype(bf16)
    obi = np.asarray(bilinear_b, f32)[np.arange(128) % O]  # bb[p%8]
    D1x_c.view(np.uint16)[:, _OBI:_OBI + 2] = obi.reshape(-1, 1).view('<u2')

    # D1b per batch: xlhT rows 64:128, col t = x_l[b, t-1]
    D1bs = []
    for b in range(B):
        D1b = np.zeros((128, _D1BW), bf16)
        D1b[NIN:, 1:1 + N] = x_l[b].T.astype(bf16)
        D1bs.append(D1b)

    in_maps = []
    for core in range(N_CORES):
        b, jg = core // 4, core % 4
        j0 = jg * JC
        D1x = D1x_c.copy()
        D1x[:NIN, _XLJ:_XLJ + JC] = x_l[b, j0:j0 + JC].T.astype(bf16)
        # xrhT: col t = x_r[b, j0-1+t], zero-padded at global edges
        lo = max(j0 - 1, 0)
        hi = min(j0 + JC + 1, N)
        D1x[:NIN, _XRH + lo - (j0 - 1):_XRH + hi - (j0 - 1)] = \
            x_r[b, lo:hi].T.astype(bf16)
        in_maps.append({
            "Dh": np.concatenate([D1w, D1x, D1bs[b]], axis=1),
            "Wt": WT,
        })
    return in_maps


def _run(inputs, trace=False, **kw):
    nc = _get_nc()
    in_maps = _prep_core_inputs(**inputs)
    res = run_bass_kernel_spmd(
        nc, in_maps, core_ids=list(range(N_CORES)), trace=trace, **kw)
    out = np.empty((B, N, N, O), np.float32)
    for core in range(N_CORES):
        b, jg = core // 4, core % 4
        j0 = jg * JC
        # device out: [p = jr*8+o, c*512 + i] -> out[i, 16c+jr, o]
        r = res.results[core]
        arr = np.concatenate(
            [np.asarray(r[f"out{i}"]) for i in range(5)],
            axis=1).astype(np.float32)
        arr = arr.reshape(16, 8, 8, N)          # [jr, o, c, i]
        out[b, :, j0:j0 + JC, :] = \
            arr.transpose(3, 2, 0, 1).reshape(N, JC, O)
    return out, res


def kernel(**inputs):
    out, _ = _run(inputs, trace=False)
    return out
